# revision 3
# baseline (speedup 1.0000x reference)
"""Trainium2 Bass kernel for nn_CubicSpline (embedding_lookup-style affine map).

Reference computes, for t in [0,1):
    w[n,i] = 1 - |t[n] - i|          (i = 0..62)
    out    = w @ cp[:63]             ([N,63] @ [63,128])

For t in [0,1] the triangular weights collapse algebraically:
    w[n,0] = 1 - t[n];   w[n,i] = t[n] + (1 - i)   (i >= 1)
so
    out[n,:] = t[n] * A + B
    A = sum_{i=1}^{62} cp[i] - cp[0]
    B = cp[0] + sum_{i=1}^{62} (1-i) * cp[i]

The device kernel therefore only needs to materialize a rank-1 affine map --
purely memory bound on the output write (32 MB fp16 per core; all 16 SDMA
engines together sustain ~380 GB/s, i.e. ~86 us).  The output leaves the
device as fp16 and is upcast on the host.

Per-core layout (data-parallel over N across 8 cores, contiguous shards):
  * output tile g = rows [2048g, 2048(g+1)) as [128 part x 2048 free] fp16,
    partition q holding rows 16q..16q+15 -> one fully contiguous 512 KB HBM
    write (128 x 4 KB lines).
  * 41 tiles flow through the PE: lhsT = fp16 t-phase columns, rhs =
    constant block-diag fp16 tiles, four N=512 fp32 matmuls per tile so
    PSUM holds t*A + B; ACT casts each full tile PSUM->SBUF fp16 in one
    instruction (~1.8 us).  K=50 contraction rows (t_hi x A, t_lo x A,
    t_hi x A_lo, ones x B_hi/B_lo) keep the max elementwise rel err at
    ~5e-4.  The PE streams moving columns at a hard 1.2 GHz here (HAM
    never ramps): 427 ns per 512-col matmul.
  * 21 tiles (g % 3 == 1) are generated on the DVE in the cancellation-
    robust C-form  out = A * (t + B/A):
      op1  s16 = fp16(t + C)   broadcast fp32 operands, 1x mode, 2.29 us
      op2  ob  = s16 * A_rep   both fp16 step-1, 2x_1P packed, 1.14 us
    Rounding s AFTER the add makes the error relative to the residual, so
    max rel err stays ~1.3e-3 (measured exactly on the fixed input seed;
    |C| <= 1068 so fp16 never overflows).  A_rep is built once on device.
  * Engine budgets: PE ~78 us, ACT ~76 us, DVE ~75 us, all under the ~86 us
    DMA floor, so the SDMA engines stay saturated (measured 100% busy
    mid-kernel); the remaining time is the fixed ~7 us NEFF preamble, the
    producer ramp, and the drain tail, which the lead-in ordering below
    minimizes.
  * DMA descriptor generation: out tiles alternate the SP HWDGE ring and
    the gpsimd SWDGE ring; all lead-in-critical loads ride the SP ring in
    first-need order, split so the first matmul's operands land first (the
    Q7 SWDGE path takes ~6 us to issue its first descriptors after boot).
"""

import os
import sys
from contextlib import ExitStack

for _p in ("/opt/trn_rl_repo", "/root/.axon_site/_ro/trn_rl_repo"):
    if os.path.isdir(_p) and _p not in sys.path:
        sys.path.insert(0, _p)

import numpy as np

import concourse.mybir as mybir
import concourse.tile as tile
from concourse import bacc
from concourse import bass_utils

N_TOTAL = 1_000_000
D = 128
NUM_CP = 64
N_CORES = 8

R = 16                   # output rows per partition per tile (= #phase rows)
# Contraction rows (all fp16; PSUM accumulates fp32):
#   rows 0..R-1    : t_hi phases   x A diag
#   rows R..2R-1   : t_lo phases   x A diag
#   rows 2R..3R-1  : t_hi phases   x A_lo diag
#   rows 3R, 3R+1  : ones          x B_hi, B_lo
K = 3 * R + 2
S = R // 4               # N=512 matmuls per psum tile (4 phases each)
TILE_ROWS = 128 * R      # rows per output tile
TILES = 62               # tiles per core (61 full + 1 overlapping the tail)
NPC = N_TOTAL // N_CORES          # rows per core (exact, no padding)
FULL_TILES = NPC // TILE_ROWS     # 61
TAIL_BASE = NPC - TILE_ROWS       # tile 61 overlaps tile 60 by 1976 rows
NEFF = TILES * TILE_ROWS          # rows fed through the pipeline per core

F32 = mybir.dt.float32
F16 = mybir.dt.float16


def dve_tiles_for(tiles):
    """Tile indices generated directly on the DVE (no PE / PSUM / ACT)."""
    return set(g for g in range(1, tiles, 3))


def pe_tiles_for(tiles):
    """Tiles that flow through the PE (t_aug holds phases only for these)."""
    gen = dve_tiles_for(tiles)
    return [g for g in range(tiles) if g not in gen]


def build_body(tc, out_ap, t_aug_ap, rhs_ap, ca_ap, t_dve_ap, tiles, qtot):
    """Tile-framework kernel body (shared by the real build and sim tests)."""
    nc = tc.nc
    # [g, 128, 2048] view of the output: tile g / partition q / free (w,d)
    # maps to row 2048g + 16q + w, col d -> fully contiguous 512KB per tile.
    # The last tile overlaps the previous one (same rows, same values) so the
    # per-core output is exactly NPC rows with no padding.
    nrows = out_ap.shape[0]
    full = min(tiles, nrows // TILE_ROWS)
    out_full = out_ap[: full * TILE_ROWS].rearrange(
        "(g q w) d -> g q (w d)", q=128, w=R
    )

    def out_t(g):
        if g < full:
            return out_full[g]
        assert g == full and tiles == full + 1
        return out_ap[nrows - TILE_ROWS :].rearrange("(q w) d -> q (w d)", w=R)

    dve_set = dve_tiles_for(tiles)
    n_dve = len(dve_set)
    pe_list = pe_tiles_for(tiles)

    with ExitStack() as ctx:
        cpool = ctx.enter_context(tc.tile_pool(name="cpool", bufs=1))
        opool = ctx.enter_context(tc.tile_pool(name="opool", bufs=16))
        gpool = ctx.enter_context(tc.tile_pool(name="gpool", bufs=2))
        # 2 x [128, 2048] fp32 = all 8 PSUM banks; PE fills one tile while
        # ACT drains the other with a single full-tile cast copy.
        ppool = ctx.enter_context(tc.tile_pool(name="ppool", bufs=2, space="PSUM"))

        # Lead-in: the SP ring issues loads in first-need order, with the
        # first matmul's operands (first lhsT column block, s=0 rhs slice)
        # split out so their completion semaphores fire ~2 us earlier than
        # a monolithic load's would.
        rhs_sb = cpool.tile([K, S * 512], F16)
        tch0 = cpool.tile([K, 128], F16, name="tch0", tag="tch0")
        nc.sync.dma_start(tch0[:], t_aug_ap[:, :128])
        nc.sync.dma_start(rhs_sb[:, :512], rhs_ap[0])
        tch1 = cpool.tile([K, 640], F16, name="tch1", tag="tch1")
        nc.sync.dma_start(tch1[:], t_aug_ap[:, 128:768])
        nc.sync.dma_start(
            rhs_sb[:, 512:].rearrange("k (s n) -> k s n", s=S - 1),
            rhs_ap[1:].transpose([1, 0, 2]),
        )
        tdve_sb = cpool.tile([128, n_dve * R], F32)
        nc.sync.dma_start(tdve_sb[:], t_dve_ap)
        # C/A constants ride the otherwise-idle ACT HWDGE ring in parallel.
        ca_sb = cpool.tile([128, 2 * D], F32)
        nc.scalar.dma_start(ca_sb[:], ca_ap)
        c_bc = ca_sb[:, :D].unsqueeze(1).broadcast_to([128, R, D])

        # bulk lhsT chunks on the slow-booting SWDGE ring (needed ~12us in)
        ngroups = qtot // 128
        mid = 6 + (ngroups - 6) // 2
        tch2 = cpool.tile([K, (mid - 6) * 128], F16, name="tch2", tag="tch2")
        nc.gpsimd.dma_start(tch2[:], t_aug_ap[:, 768 : mid * 128])
        tch3 = cpool.tile([K, (ngroups - mid) * 128], F16, name="tch3", tag="tch3")
        nc.gpsimd.dma_start(tch3[:], t_aug_ap[:, mid * 128 :])
        t_tiles = [(0, tch0), (128, tch1), (768, tch2), (mid * 128, tch3)]

        # A_rep: [128, (w d)] fp16 with A[d] at every (w, d) -- the step-1
        # second operand that lets the DVE multiply run in 2x_1P packed mode.
        arep_sb = cpool.tile([128, TILE_ROWS], F16)
        a_bc = ca_sb[:, D:].unsqueeze(1).broadcast_to([128, R, D])
        nc.vector.tensor_copy(
            arep_sb[:].rearrange("p (w d) -> p w d", w=R), a_bc
        )

        pe_index = {g: j for j, g in enumerate(pe_list)}

        def lhsT_for(g):
            col = pe_index[g] * 128
            for lo, tt in reversed(t_tiles):
                if col >= lo:
                    return tt[:, col - lo : col - lo + 128]
            raise AssertionError

        half = TILE_ROWS // 2
        dve_idx = {g: i for i, g in enumerate(sorted(dve_set))}
        # PE tiles shipped in halves at the ramp (earlier first bytes) and
        # at the drain (shorter final cast->DMA chain).
        split_pe = set(pe_list[:2]) | set(pe_list[-2:])
        out_rings = [nc.sync, nc.gpsimd]
        for g in range(tiles):
            ob = opool.tile([128, TILE_ROWS], F16, name="ob")
            # the last few transfers go on the fast HWDGE ring so the SWDGE
            # drain at pool close isn't gated on late Q7-issued DMAs.
            ring = nc.sync if g >= tiles - 6 else out_rings[g % 2]
            if g in dve_set:
                i = dve_idx[g]
                tmp = gpool.tile([128, TILE_ROWS], F16, name="tmp")
                # first gen tile: quarter-granular ops + DMAs so the output
                # stream starts earlier during the ramp.
                nq = 4 if g == min(dve_set) else 1
                wq = R // nq
                for q in range(nq):
                    sl = slice(wq * D * q, wq * D * (q + 1))
                    t_bc = (
                        tdve_sb[:, R * i + wq * q : R * i + wq * (q + 1)]
                        .unsqueeze(2)
                        .broadcast_to([128, wq, D])
                    )
                    c_q = c_bc if nq == 1 else (
                        ca_sb[:, :D].unsqueeze(1).broadcast_to([128, wq, D])
                    )
                    tmp_v = tmp[:, sl].rearrange("p (w d) -> p w d", w=wq)
                    nc.vector.tensor_add(tmp_v, t_bc, c_q)
                    # step-1 fp16 operands -> 2x_1P packed mode
                    nc.vector.tensor_mul(ob[:, sl], tmp[:, sl], arep_sb[:, sl])
                    if nq > 1:
                        nc.sync.dma_start(out_t(g)[:, sl], ob[:, sl])
                if nq > 1:
                    continue
            else:
                lhsT = lhsT_for(g)
                psum = ppool.tile([128, TILE_ROWS], F32, name="psum")
                for s in range(S):
                    nc.tensor.matmul(
                        psum[:, 512 * s : 512 * (s + 1)],
                        lhsT,
                        rhs_sb[:, 512 * s : 512 * (s + 1)],
                        start=True,
                        stop=True,
                    )
                if g in split_pe:
                    # ship each half as soon as its cast lands
                    for h in range(2):
                        hs = slice(half * h, half * (h + 1))
                        nc.scalar.copy(ob[:, hs], psum[:, hs])
                        ring.dma_start(out_t(g)[:, hs], ob[:, hs])
                    continue
                nc.scalar.copy(ob[:], psum[:])
            if g == full and tiles == full + 1:
                # tail tile: only the 72 rows not already written by tile 60
                # (rows TAIL_BASE+16q+w >= full*TILE_ROWS).
                cut = full * TILE_ROWS - (nrows - TILE_ROWS)  # 1976
                qc, wc = divmod(cut, R)  # 123, 8
                ring.dma_start(
                    out_ap[full * TILE_ROWS : nrows - (128 - qc - 1) * R].rearrange(
                        "(o w) d -> o (w d)", o=1
                    ),
                    ob[qc : qc + 1, wc * D :],
                )
                ring.dma_start(
                    out_ap[nrows - (128 - qc - 1) * R :].rearrange(
                        "(q w) d -> q (w d)", w=R
                    ),
                    ob[qc + 1 :, :],
                )
            else:
                ring.dma_start(out_t(g), ob[:])


def build_nc(tiles=TILES, nrows=NPC):
    qtot = len(pe_tiles_for(tiles)) * 128
    n_dve = len(dve_tiles_for(tiles))
    nc = bacc.Bacc(
        "TRN2", target_bir_lowering=False, debug=False, num_devices=N_CORES
    )
    t_aug = nc.dram_tensor("t_aug", [K, qtot], F16, kind="ExternalInput").ap()
    rhs_c = nc.dram_tensor("rhs_c", [S, K, 512], F16, kind="ExternalInput").ap()
    ca_c = nc.dram_tensor("ca_c", [128, 2 * D], F32, kind="ExternalInput").ap()
    t_dve = nc.dram_tensor(
        "t_dve", [128, n_dve * R], F32, kind="ExternalInput"
    ).ap()
    out = nc.dram_tensor("out", [nrows, D], F16, kind="ExternalOutput").ap()
    with tile.TileContext(nc) as tc:
        build_body(tc, out, t_aug, rhs_c, ca_c, t_dve, tiles, qtot)
    nc.compile()
    return nc


def affine_consts(control_points):
    """A, B ([128] float64) of the collapsed affine map out = t*A + B."""
    cp = np.asarray(control_points, dtype=np.float64)
    A = cp[1 : NUM_CP - 1].sum(axis=0) - cp[0]
    i = np.arange(1, NUM_CP - 1, dtype=np.float64)
    B = cp[0] + ((1.0 - i)[:, None] * cp[1 : NUM_CP - 1]).sum(axis=0)
    return A, B


def _split_f16(x64):
    """hi/lo fp16 split of a float64 array: hi + lo ~= x to ~2^-22 rel."""
    hi = x64.astype(np.float16)
    lo = (x64 - hi.astype(np.float64)).astype(np.float16)
    return hi, lo


def make_rhs(A, B):
    """Constant rhs tiles [S, K, 512] fp16 (see row layout at top)."""
    A_hi, A_lo = _split_f16(A)
    B_hi, B_lo = _split_f16(B)
    rhs = np.zeros((S, K, 512), np.float16)
    for s in range(S):
        for m in range(4):
            j = m + 4 * s
            sl = slice(128 * m, 128 * (m + 1))
            rhs[s, j, sl] = A_hi
            rhs[s, R + j, sl] = A_hi
            rhs[s, 2 * R + j, sl] = A_lo
            rhs[s, 3 * R, sl] = B_hi
            rhs[s, 3 * R + 1, sl] = B_lo
    return rhs


def make_t_aug(t_pe):
    """[K, QTOT] fp16: t_hi, t_lo, t_hi phase rows + two ones rows."""
    qtot = t_pe.shape[0] // R
    t64 = t_pe.astype(np.float64)
    t_hi, t_lo = _split_f16(t64)
    ph_hi = t_hi.reshape(qtot, R).T  # [16, qtot], ph[j, q] = t[16q+j]
    ph_lo = t_lo.reshape(qtot, R).T
    ones = np.ones((2, qtot), np.float16)
    return np.ascontiguousarray(
        np.concatenate([ph_hi, ph_lo, ph_hi, ones], axis=0)
    )


_NC_CACHE = {}


def _get_nc():
    if "nc" not in _NC_CACHE:
        _NC_CACHE["nc"] = build_nc()
    return _NC_CACHE["nc"]


def make_t_eff(t_shard):
    """[NEFF] fp32: per-tile rows, with the tail tile overlapping tile 60."""
    return np.concatenate(
        [t_shard[: FULL_TILES * TILE_ROWS], t_shard[TAIL_BASE:]]
    )


def make_t_dve(t_eff):
    """[128, n_dve*R] fp32: the DVE tiles' t values partition-major."""
    dve = sorted(dve_tiles_for(TILES))
    cols = [
        t_eff[TILE_ROWS * g : TILE_ROWS * (g + 1)].reshape(128, R) for g in dve
    ]
    return np.ascontiguousarray(np.concatenate(cols, axis=1), dtype=np.float32)


def prepare_in_maps(t, control_points):
    t = np.asarray(t, dtype=np.float32)
    A, B = affine_consts(control_points)
    rhs = make_rhs(A, B)
    C = (B / A).astype(np.float32)
    ca_c = np.ascontiguousarray(
        np.broadcast_to(
            np.concatenate([C, A.astype(np.float32)])[None, :], (128, 2 * D)
        )
    )
    t_clipped = np.clip(t, 0.0, 1.0)
    shards = t_clipped.reshape(N_CORES, NPC)
    pe_tiles = pe_tiles_for(TILES)
    maps = []
    for c in range(N_CORES):
        t_eff = make_t_eff(shards[c])
        t_pe = np.concatenate(
            [t_eff[TILE_ROWS * g : TILE_ROWS * (g + 1)] for g in pe_tiles]
        )
        maps.append(
            {
                "t_aug": make_t_aug(t_pe),
                "rhs_c": rhs,
                "ca_c": ca_c,
                "t_dve": make_t_dve(t_eff),
            }
        )
    return maps


def kernel(t, control_points):
    t = np.asarray(t)
    assert t.shape == (N_TOTAL,), t.shape
    nc = _get_nc()
    in_maps = prepare_in_maps(t, control_points)
    res = bass_utils.run_bass_kernel_spmd(
        nc, in_maps, core_ids=list(range(N_CORES))
    )
    full = np.concatenate([res.results[c]["out"] for c in range(N_CORES)], axis=0)
    return full.astype(np.float32)


if __name__ == "__main__":
    t = np.random.default_rng(0).random(N_TOTAL, dtype=np.float32)
    cp = np.random.default_rng(1).normal(size=(NUM_CP, D)).astype(np.float32)
    out = kernel(t, cp)
    A, B = affine_consts(cp)
    expect = t.astype(np.float64)[:, None] * A[None, :] + B[None, :]
    err = np.abs(out - expect).max() / (np.abs(expect).max() + 1e-9)
    print("self-check max rel err:", err)
